# revision 12
# baseline (speedup 1.0000x reference)
"""CNN self-attention kernel for Trainium2 (8 NeuronCores, data-parallel over batch).

Reference computation per batch b (C=256, Cp=32, N=64*64=4096):
    f  = relu(Wq @ x)   (Cp, N)
    g  = relu(Wk @ x)   (Cp, N)
    h  = relu(Wv @ x)   (C, N)
    S  = f^T g          (N, N)     S[n, m]
    beta = softmax(S, axis=1)
    o  = gamma * (h @ beta) + x

Kernel strategy (one batch per core):
    - Scores in fp16 (precision), E/AV path in bf16 (range), fp32 PSUM.
    - exp(S - 16) with no max pass; the shift cancels in the softmax ratio.
    - Scores via 4-way row-tiled K=32 matmuls (tile_position packing, all
      128 PE rows, ONE strip burst per 2 groups -> half the strip/full
      array switch overhead).
    - Per super-cycle (4 n-chunks x 512 m): 4 concurrent strip matmuls
      fill two 2-bank PSUM tiles; tile A is exp'd exactly on ACT (bf16
      out); tile B is exp'd on DVE with a Schraudolph fast-exp: bits =
      round(184.664*s + 13295.9) written as int16 and reinterpreted as
      bf16 gives e^(s-16) with |rel err| <= 3%.  Splitting the 16.8M-exp
      stream across both engines removes the ACT bottleneck; the hybrid
      is numerically safe (every softmax column keeps exact entries from
      the ACT half; validated: rel err 3.3e-4 vs 2e-2 budget).
    - g is replicated 4x across partition groups directly by the
      projection matmul (lhsT = [WkT|WkT|WkT|WkT]); f is scattered into
      the 4-way packed layout by tiny col-tiled identity matmuls.  DVE
      cross-partition-base copies to bases 64/96 cost ~1.9us each (slow
      path) - never use them.
    - AV matmul computed transposed: O'[m, c] = sum_n E[n, m] * hT[n, c],
      with a ones-column appended to hT so column C of O' is the softmax
      denominator D[m] for free, per-partition.
    - Epilogue split: at the m-block boundary the four AV accumulators
      are released by four plain PSUM->SBUF copies (2 on DVE, 2 on ACT)
      so the next m-block's AV matmuls are not blocked; the scale
      (gamma/D on ACT via Copy-with-scale), DMA-xbar transposes back to
      [c, m] and the fp32 residual add are spread ONE PIECE PER
      super-cycle across the following m-block — the engine queues are
      strict FIFO, and issuing the whole epilogue at once head-of-line
      blocks the per-super exp ops behind ~8us of serialized DMA
      transposes, stalling the PE at every boundary long enough for the
      HAM clock gate to re-throttle it to half rate.
      Last m-block uses PE transposes (PE is idle in the tail).
"""

import os
import sys

import numpy as np

for _p in ("/root/.axon_site/_ro/trn_rl_repo", "/opt/trn_rl_repo"):
    if os.path.isdir(_p) and _p not in sys.path:
        sys.path.append(_p)

import concourse.bacc as bacc
import concourse.mybir as mybir
import concourse.tile as tile
from concourse.bass_utils import run_bass_kernel_spmd
from concourse.masks import make_identity

P = 128
C = 256
CP = 32
N = 4096
NCH = N // P      # 32 n-chunks of 128
NB = 512          # n-block width for projections and m-block width
NMB = N // NB     # 8 m-blocks
NSUP = 64         # super-cycles: 8 m-blocks x 8 chunk-quads
SHIFT = 16.0
SCH_A = float(128.0 * np.log2(np.e))             # 184.6637
SCH_B = float(-SHIFT * SCH_A + 127 * 128 - 5.5)  # minimax bias
F16 = mybir.dt.float16
BF16 = mybir.dt.bfloat16
I16 = mybir.dt.int16
F32 = mybir.dt.float32
N_CORES = 8

_CACHE = {}


def build_nc():
    nc = bacc.Bacc("TRN2", target_bir_lowering=False, debug=False)

    x_d = nc.dram_tensor("x", (C, N), F32, kind="ExternalInput").ap()
    wq_d = nc.dram_tensor("Wq", (CP, C), F32, kind="ExternalInput").ap()
    wk_d = nc.dram_tensor("Wk", (CP, C), F32, kind="ExternalInput").ap()
    wv_d = nc.dram_tensor("Wv", (C, C), F32, kind="ExternalInput").ap()
    g_d = nc.dram_tensor("gamma", (1,), F32, kind="ExternalInput").ap()
    out_d = nc.dram_tensor("out", (C, N), F32, kind="ExternalOutput").ap()

    MM = mybir.AluOpType.mult
    ADD = mybir.AluOpType.add
    ACOPY = mybir.ActivationFunctionType.Copy
    ARELU = mybir.ActivationFunctionType.Relu
    AEXP = mybir.ActivationFunctionType.Exp

    with tile.TileContext(nc) as tc:
        with (
            tc.tile_pool(name="const", bufs=1) as constp,
            tc.tile_pool(name="big", bufs=1) as bigp,
            tc.tile_pool(name="ep", bufs=4) as ep,
            tc.tile_pool(name="fgp", bufs=2) as fgp,
            tc.tile_pool(name="pocp", bufs=4) as pocpp,
            tc.tile_pool(name="oscp", bufs=4) as oscp,
            tc.tile_pool(name="trp", bufs=4) as trp,
            tc.tile_pool(name="outp", bufs=2) as outp,
            tc.tile_pool(name="recp", bufs=8) as recp,
            # PSUM: psA = two [128,2,512] score tiles (2 banks each; one
            # super-cycle's 4-way strip burst fills both); psO = 4 x
            # [128,257] AV accumulators (1 bank each); setup/projection
            # psums borrow psA/psO slots. Exactly 8 banks.
            tc.tile_pool(name="psA", bufs=2, space="PSUM") as psA,
            tc.tile_pool(name="psO", bufs=4, space="PSUM") as psO,
        ):
            # ---- big persistent tiles ----
            X32 = bigp.tile([P, 2, N], F32)
            X16 = bigp.tile([P, 2, N], F16)
            hT = bigp.tile([P, NCH, C + 1], BF16)     # [n-chunk, c | ones]
            fpk = bigp.tile([P, NCH // 4, P], F16)    # f chunk j at partitions 32*(j%4), col j//4
            grep = bigp.tile([P, N], F16)             # g replicated 4x along partitions

            nc.vector.memset(hT[:, :, C:C + 1], 1.0)

            # ---- small weight/gamma loads first (they gate the PE setup),
            # then prefetch all of x split across two HWDGE queues ----
            gamma_sb = constp.tile([P, 1], F32)
            wq_sb = constp.tile([P, C], F32)
            nc.sync.dma_start(wq_sb[:CP, :], wq_d)
            wk_sb = constp.tile([P, C], F32)
            nc.sync.dma_start(wk_sb[:CP, :], wk_d)
            wv_sb = constp.tile([P, 2, C], F32)
            nc.sync.dma_start(wv_sb, wv_d.rearrange("(k p) c -> p k c", p=P))

            # x prefetch round-robined whole-block across the three DMA
            # queues (sync/scalar HWDGE + gpsimd SWDGE) — on two queues the
            # x load paces the projections (last block lands ~25us in).
            x_engs = (nc.sync, nc.scalar, nc.gpsimd)
            for nb in range(N // NB):
                sl = slice(nb * NB, (nb + 1) * NB)
                eng = x_engs[nb % 3]
                eng.dma_start(X32[:, 0, sl], x_d[0:P, sl])
                eng.dma_start(X32[:, 1, sl], x_d[P:2 * P, sl])

            # gamma broadcast is 128 tiny SWDGE descriptors — queue it after
            # the x blocks (only needed at the first epilogue, ~60us in).
            nc.gpsimd.dma_start(out=gamma_sb, in_=g_d.to_broadcast((P, 1)))

            # ---- constants ----
            ident = constp.tile([P, P], F32)
            make_identity(nc, ident)

            shift_sb = constp.tile([P, 1], F32)
            nc.vector.memset(shift_sb, -SHIFT)

            ident16 = constp.tile([CP, CP], F16)
            nc.vector.tensor_copy(ident16, ident[0:CP, 0:CP])

            # Wq/Wk staged zero-padded to 128 partitions so the PE transposes
            # below are standard full-height [128,128] transposes.
            # partition ranges must be 32-aligned blocks (base in {0,32,64,96},
            # count <= 32 unless base-aligned larger) — split the zero-fills.
            nc.vector.memset(wq_sb[CP:2 * CP, :], 0.0)
            nc.vector.memset(wq_sb[2 * CP:, :], 0.0)
            nc.vector.memset(wk_sb[CP:2 * CP, :], 0.0)
            nc.vector.memset(wk_sb[2 * CP:, :], 0.0)

            # wqT[:, cc, :] = Wq[:, cc*128:+128]^T (for the f projection);
            # wgrep[:, cc, 32r:+32] = Wk[:, cc*128:+128]^T x4 (the g
            # projection directly produces g replicated across the four
            # partition groups).  4 transposes back-to-back, then copies.
            wqT = constp.tile([P, 2, CP], F16)
            wgrep = constp.tile([P, 2, P], F16)
            ptqk = []
            for cc in range(2):
                ptq = psO.tile([P, C + 1], F32, tag="o", name=f"ptq{cc}")
                nc.tensor.transpose(
                    ptq[:, :P], wq_sb[:, cc * P:(cc + 1) * P], ident
                )
                ptk = psO.tile([P, C + 1], F32, tag="o", name=f"ptk{cc}")
                nc.tensor.transpose(
                    ptk[:, :P], wk_sb[:, cc * P:(cc + 1) * P], ident
                )
                ptqk.append((ptq, ptk))
            for cc in range(2):
                ptq, ptk = ptqk[cc]
                nc.vector.tensor_copy(wqT[:, cc, :], ptq[:, :CP])
                for r in range(4):
                    nc.vector.tensor_copy(
                        wgrep[:, cc, 32 * r:32 * (r + 1)], ptk[:, :CP]
                    )

            # wvT[:, cc, mc*128:+128] = Wv[mc*128:+128, cc*128:+128]^T
            wvT = constp.tile([P, 2, C], F16)
            ptvs = []
            for cc in range(2):
                for mc in range(2):
                    ptv = psO.tile([P, C + 1], F32, tag="o", name=f"ptv{cc}{mc}")
                    nc.tensor.transpose(
                        ptv[:, :P], wv_sb[:, mc, cc * P:(cc + 1) * P], ident
                    )
                    ptvs.append((cc, mc, ptv))
            for cc, mc, ptv in ptvs:
                nc.vector.tensor_copy(wvT[:, cc, mc * P:(mc + 1) * P], ptv[:, :P])

            # ---- projections (per 512-wide n-block, paced by the x DMAs) ----
            for nb in range(N // NB):
                sl = slice(nb * NB, (nb + 1) * NB)
                nc.vector.tensor_copy(X16[:, 0, sl], X32[:, 0, sl])
                nc.scalar.copy(X16[:, 1, sl], X32[:, 1, sl])

                # g replicated: [WkT x4] @ X -> [128, 512]; f: WqT @ X -> [32, 512]
                psp = psA.tile([P, 2, NB], F32, tag="s", name=f"psp{nb}")
                for cc in range(2):
                    nc.tensor.matmul(
                        psp[:, 0, :], wgrep[:, cc, :], X16[:, cc, sl],
                        start=(cc == 0), stop=(cc == 1),
                    )
                for cc in range(2):
                    nc.tensor.matmul(
                        psp[:CP, 1, :], wqT[:, cc, :], X16[:, cc, sl],
                        start=(cc == 0), stop=(cc == 1),
                    )
                if nb % 2 == 0:
                    nc.scalar.activation(grep[:, sl], psp[:, 0, :], ARELU)
                else:
                    nc.vector.tensor_scalar_max(grep[:, sl], psp[:, 0, :], 0.0)
                f16v = fgp.tile([CP, NB], F16, tag="f16", name=f"f16_{nb}")
                nc.vector.tensor_scalar_max(f16v, psp[:CP, 1, :], 0.0)

                # f scatter: 4 col-tiled identity matmuls put chunk 4nb+u at
                # partitions 32u; one full-partition DVE drain into fpk.
                fps = psO.tile([P, C + 1], F32, tag="o", name=f"fps{nb}")
                for u in range(4):
                    nc.tensor.matmul(
                        fps[32 * u:32 * (u + 1), :P],
                        ident16,
                        f16v[:, u * P:(u + 1) * P],
                        start=True, stop=True,
                        tile_position=(0, 32 * u),
                    )
                nc.vector.tensor_copy(fpk[:, nb, :], fps[:, :P])

                # hT: X^T @ Wv^T -> [n 128, c 256] in bf16; 4 chunks per psA
                # tile so the relu+cast drains 4 chunks in one strided op.
                psh = psA.tile([P, 2, NB], F32, tag="s", name=f"psh{nb}")
                for u in range(4):
                    j = 4 * nb + u
                    phv = psh[:, u // 2, (u % 2) * C:(u % 2 + 1) * C]
                    for cc in range(2):
                        nc.tensor.matmul(
                            phv, X16[:, cc, j * P:(j + 1) * P], wvT[:, cc, :],
                            start=(cc == 0), stop=(cc == 1),
                        )
                if nb % 2 == 0:
                    nc.vector.tensor_scalar_max(
                        hT[:, 4 * nb:4 * (nb + 1), 0:C],
                        psh.rearrange("p a (b c) -> p (a b) c", c=C),
                        0.0,
                    )
                else:
                    nc.scalar.activation(
                        hT[:, 4 * nb:4 * (nb + 1), 0:C],
                        psh.rearrange("p a (b c) -> p (a b) c", c=C),
                        ARELU,
                    )

            # ---- main attention loop over super-cycles ----
            # super t = (m-block t//8, chunk-quad t%8): 4 strip matmuls fill
            # psum tiles A (chunks 4q,4q+1) and B (4q+2,4q+3); exp A on ACT,
            # exp B on DVE (Schraudolph); AV for super t-1 overlaps.
            po = [None] * 4
            Ets = {}
            pocps = {}

            def scores(t):
                mb, q = t // 8, t % 8
                msl = slice(mb * NB, (mb + 1) * NB)
                pa = psA.tile([P, 2, NB], F32, tag="s", name=f"pa{t}")
                pb = psA.tile([P, 2, NB], F32, tag="s", name=f"pb{t}")
                for r in range(4):
                    dst = pa[:, r, :] if r < 2 else pb[:, r - 2, :]
                    nc.tensor.matmul(
                        dst,
                        fpk[32 * r:32 * (r + 1), q, :],
                        grep[32 * r:32 * (r + 1), msl],
                        start=True, stop=True,
                        tile_position=(32 * r, 0),
                    )
                return pa, pb

            def expsup(t, pa, pb):
                Ea = ep.tile([P, 2, NB], BF16, tag="et", name=f"ea{t}")
                nc.scalar.activation(
                    Ea[:, :, :], pa[:, :, :], AEXP,
                    bias=shift_sb[:, :], scale=1.0,
                )
                Eb = ep.tile([P, 2, NB], BF16, tag="et", name=f"eb{t}")
                nc.vector.tensor_scalar(
                    Eb.bitcast(I16)[:, :, :], pb[:, :, :],
                    SCH_A, SCH_B, MM, ADD,
                )
                Ets[t] = (Ea, Eb)

            def av(t):
                mb, q = t // 8, t % 8
                if q == 0:
                    for i in range(4):
                        po[i] = psO.tile(
                            [P, C + 1], F32, tag="o", name=f"po{i}_{mb}"
                        )
                Ea, Eb = Ets.pop(t)
                for half, Et in ((0, Ea), (1, Eb)):
                    for jl in range(2):
                        j = 4 * q + 2 * half + jl
                        for mc in range(4):
                            nc.tensor.matmul(
                                po[mc][:, :],
                                Et[:, jl, mc * P:(mc + 1) * P],
                                hT[:, j, :],
                                start=(j == 0), stop=(j == NCH - 1),
                                skip_group_check=True,
                            )

            def epilogue_release(mb):
                # free the AV accumulators ASAP: 2 plain copies on DVE + 2 on
                # ACT; everything downstream reads the SBUF copies.
                cps = []
                for mc in range(4):
                    pc = pocpp.tile([P, C + 1], F32, tag="pc", name=f"pc{mc}_{mb}")
                    cps.append(pc)
                nc.vector.tensor_copy(cps[0], po[0][:, :])
                nc.scalar.copy(cps[1], po[1][:, :])
                nc.vector.tensor_copy(cps[2], po[2][:, :])
                nc.scalar.copy(cps[3], po[3][:, :])
                pocps[mb] = cps

            # Epilogue for m-blocks 0..6 is SPREAD one small piece per
            # super-cycle over the next m-block: the DVE/ACT/sync queues are
            # strict FIFO, and dumping 8 serialized DMA transposes + 8 DVE
            # adds at once head-of-line-blocks the per-super exp ops, which
            # stalls the PE ~7us per boundary (and lets the HAM clock gate
            # re-throttle the PE to half rate).  One transpose + one add per
            # super hides entirely under the per-super slack.
            oscs = {}
            trs = {}
            osts = {}

            def epi_scale(mb, mc):
                # recip/rec2 on DVE (tiny); the gamma/D scale runs on the
                # otherwise-idle GPSIMD engine so it neither delays Ea on ACT
                # nor sits ahead of the next exp in the DVE FIFO.
                pc = pocps[mb][mc]
                rec = recp.tile([P, 1], F32, tag="rec", name=f"rec{mc}_{mb}")
                nc.vector.reciprocal(rec, pc[:, C:C + 1])
                rec2 = recp.tile([P, 1], F32, tag="rec2", name=f"rec2_{mc}_{mb}")
                nc.vector.tensor_scalar_mul(rec2, rec, gamma_sb)
                osc = oscp.tile([P, C], F16, tag="osc", name=f"osc{mc}_{mb}")
                nc.scalar.activation(osc, pc[:, 0:C], ACOPY, scale=rec2)
                oscs[(mb, mc)] = osc

            def epi_tr(mb, mc, cc):
                osc = oscs[(mb, mc)]
                tr = trp.tile([P, P], F16, tag="tr", name=f"tr{mc}{cc}_{mb}")
                nc.sync.dma_start_transpose(tr, osc[:, cc * P:(cc + 1) * P])
                trs[(mb, mc, cc)] = tr

            def epi_add(mb, mc, cc):
                # residual add on GPSIMD: it may wait on its DMA transpose
                # without head-of-line-blocking the DVE exp stream.
                tr = trs.pop((mb, mc, cc))
                nc.vector.tensor_tensor(
                    osts[mb][:, cc, mc * P:(mc + 1) * P],
                    tr,
                    X32[:, cc, mb * NB + mc * P: mb * NB + (mc + 1) * P],
                    ADD,
                )

            def epi_store(mb):
                ost = osts.pop(mb)
                pocps.pop(mb)
                msl = slice(mb * NB, (mb + 1) * NB)
                for cc in range(2):
                    nc.sync.dma_start(out_d[cc * P:(cc + 1) * P, msl], ost[:, cc, :])

            sched = {}

            def at_body(t, fn):
                sched.setdefault(t, []).append(fn)

            def schedule_epilogue(mb, tb):
                # per body: at most one scale, one transpose, one add; the
                # add trails its transpose by TWO bodies so it never waits.
                def step(s):
                    def fn():
                        if s == 0:
                            osts[mb] = outp.tile(
                                [P, 2, NB], F32, tag="ost", name=f"ost{mb}"
                            )
                        if s % 2 == 0:
                            epi_scale(mb, s // 2)
                        epi_tr(mb, s // 2, s % 2)
                        if s >= 2:
                            epi_add(mb, (s - 2) // 2, (s - 2) % 2)
                    return fn

                for s in range(8):
                    at_body(tb + 1 + s, step(s))

                def tail1():
                    epi_add(mb, 3, 0)

                def tail2():
                    epi_add(mb, 3, 1)
                    epi_store(mb)

                at_body(tb + 9, tail1)
                at_body(tb + 10, tail2)

            def epilogue_finish_last(mb):
                # final m-block: PE is idle in the tail, so transpose there.
                msl = slice(mb * NB, (mb + 1) * NB)
                cps = pocps.pop(mb)
                ost = outp.tile([P, 2, NB], F32, tag="ost", name="ost_last")
                for mc in range(4):
                    pc = cps[mc]
                    rec = recp.tile([P, 1], F32, name=f"rec{mc}")
                    nc.vector.reciprocal(rec, pc[:, C:C + 1])
                    rec2 = recp.tile([P, 1], F32, name=f"rec2_{mc}")
                    nc.vector.tensor_scalar_mul(rec2, rec, gamma_sb)
                    osc = oscp.tile([P, C], F16, tag="osc", name=f"oscL{mc}")
                    nc.scalar.activation(osc, pc[:, 0:C], ACOPY, scale=rec2)
                    osc32 = oscp.tile([P, C], F32, tag="osc32", name=f"o32_{mc}")
                    nc.vector.tensor_copy(osc32, osc)
                    for cc in range(2):
                        pt = psA.tile([P, 2, NB], F32, tag="s", name=f"pt{mc}{cc}")
                        ptv = pt.rearrange("p a b -> p (a b)")[:, :P]
                        nc.tensor.transpose(
                            ptv, osc32[:, cc * P:(cc + 1) * P], ident
                        )
                        nc.vector.tensor_tensor(
                            ost[:, cc, mc * P:(mc + 1) * P],
                            ptv,
                            X32[:, cc, mb * NB + mc * P: mb * NB + (mc + 1) * P],
                            ADD,
                        )
                for cc in range(2):
                    nc.sync.dma_start(out_d[cc * P:(cc + 1) * P, msl], ost[:, cc, :])

            for t in range(NSUP + 4):
                if t < NSUP:
                    pa, pb = scores(t)
                    expsup(t, pa, pb)
                for fn in sched.pop(t, ()):
                    fn()
                if 1 <= t <= NSUP:
                    av(t - 1)
                    if (t - 1) % 8 == 7:
                        mb = (t - 1) // 8
                        epilogue_release(mb)
                        if mb < NMB - 1:
                            schedule_epilogue(mb, t)
                        else:
                            at_body(t + 1, lambda mb=mb: epilogue_finish_last(mb))
            assert not sched, f"unscheduled epilogue bodies: {list(sched)}"

    nc.compile()
    return nc


def _get_nc():
    if "nc" not in _CACHE:
        _CACHE["nc"] = build_nc()
    return _CACHE["nc"]


def _make_in_maps(inputs):
    x = np.ascontiguousarray(np.asarray(inputs["x"], dtype=np.float32))
    B = x.shape[0]
    assert B == N_CORES
    wq = np.ascontiguousarray(np.asarray(inputs["Wq"], dtype=np.float32))
    wk = np.ascontiguousarray(np.asarray(inputs["Wk"], dtype=np.float32))
    wv = np.ascontiguousarray(np.asarray(inputs["Wv"], dtype=np.float32))
    gamma = np.ascontiguousarray(np.asarray(inputs["gamma"], dtype=np.float32))
    return [
        {
            "x": x[b].reshape(C, N),
            "Wq": wq,
            "Wk": wk,
            "Wv": wv,
            "gamma": gamma,
        }
        for b in range(B)
    ]


def run(inputs, trace=False, **kwargs):
    nc = _get_nc()
    in_maps = _make_in_maps(inputs)
    res = run_bass_kernel_spmd(
        nc, in_maps, core_ids=list(range(N_CORES)), trace=trace, **kwargs
    )
    x = np.asarray(inputs["x"])
    B, Cx, H, W = x.shape
    out = np.stack([res.results[b]["out"] for b in range(B)])
    return out.reshape(B, Cx, H, W).astype(np.float32), res


def kernel(**inputs):
    out, _ = run(inputs)
    return out

